# revision 1
# baseline (speedup 1.0000x reference)
"""3-layer GCN (GCNConv + residual + relu, global add pool, MLP softmax) on 8
Trainium2 NeuronCores.

Sharding: nodes/edges partitioned by destination-node range across the 8
cores. Per layer: each core computes its shard of xw' = (D^-1/2 h) @ Wg,
AllGathers the message table (bf16), then gathers per-edge message rows with
dma_gather and segment-sums them into per-128-dst-window PSUM accumulators
via one-hot matmuls (sel built on-device with is_equal against an iota).
All cores run the IDENTICAL program; per-core variation lives entirely in
data (gather indices, sel values, padding). Pooled [64,128] partials are
AllReduced and the tiny classifier is replicated.
"""
import math
import numpy as np
import ml_dtypes

import concourse.bacc as bacc
import concourse.bass as bass
import concourse.mybir as mybir
import concourse.tile as tile
from concourse.bass_utils import run_bass_kernel_spmd

NCORES = 8
G = 64    # graphs in batch (pooled rows)
C = 2     # classes
SBW = 8   # dst windows per superblock (psum granularity)
SLAB = 32  # sel tiles built per is_equal op

bf16 = ml_dtypes.bfloat16
_cache = {}


def _ceil(a, b):
    return -(-a // b)


def _ceilarr(a, b):
    return -(-a // b)


# --------------------------------------------------------------------------
# host preprocessing
# --------------------------------------------------------------------------
def _preprocess(x, edge_index, batch):
    N, D = x.shape
    assert D == 128 and N % NCORES == 0
    NLOC = N // NCORES
    NPAD = _ceil(NLOC, 128) * 128
    NW = NPAD // 128
    NSB = _ceil(NW, SBW)

    # source quarters (for split AllGather + int16 chunked gather tables)
    NW4 = _ceil(NW, 4)
    qb = [min(q * NW4, NW) for q in range(5)]      # quarter boundaries in tiles
    NWq = [qb[q + 1] - qb[q] for q in range(4)]
    CHR = [NCORES * 128 * NWq[q] for q in range(4)]  # rows per chunk table
    assert max(CHR) <= 32768

    src = np.asarray(edge_index[0], np.int64)
    dst = np.asarray(edge_index[1], np.int64)
    deg = np.bincount(dst, minlength=N).astype(np.float64) + 1.0
    dinv = (deg ** -0.5).astype(np.float32)

    loops = np.arange(N, dtype=np.int64)
    src_f = np.concatenate([src, loops])
    dst_f = np.concatenate([dst, loops])

    core = dst_f // NLOC
    dl = dst_f - core * NLOC
    sc = src_f // NLOC
    sl = src_f - sc * NLOC
    p_s = sl % 128
    a_s = sl // 128
    ch = np.minimum(a_s // NW4, 3)
    nwq_arr = np.asarray(NWq, np.int64)
    qb_arr = np.asarray(qb[:4], np.int64)
    # row within chunk table ch: rank block [128, NWq, 128] flat
    srow = sc * (128 * nwq_arr[ch]) + p_s * nwq_arr[ch] + (a_s - qb_arr[ch])
    w = dl // 128
    cell = w * 4 + ch                        # per-core cell id

    key = core * (NW * 4) + cell
    counts = np.bincount(key, minlength=NCORES * NW * 4).reshape(NCORES, NW * 4)
    import os
    _cgran = int(os.environ.get("CELL_GRAN", "32"))
    cmax = counts.max(axis=0)                # [NW*4]
    cap = _ceilarr(cmax, _cgran) * _cgran    # slots per cell (0 if empty)

    # stream layout: sb -> ch -> w; groups padded to x128
    cell_slot_off = np.zeros(NW * 4, np.int64)
    groups = []   # (sb, ch, tile_off, n_tiles)
    # per-slot window id (for tile->window mapping), -1 = group pad
    slot_w_list = []
    so = 0
    for sb in range(NSB):
        ws = range(sb * SBW, min((sb + 1) * SBW, NW))
        for c4 in range(4):
            g_so = so
            for w_ in ws:
                cid = w_ * 4 + c4
                cell_slot_off[cid] = so
                slot_w_list.append(np.full(int(cap[cid]), w_, np.int64))
                so += int(cap[cid])
            g_slots = so - g_so
            pad = (-g_slots) % 128
            if pad:
                slot_w_list.append(np.full(pad, -1, np.int64))
                so += pad
            groups.append((sb, c4, g_so // 128, (so - g_so) // 128))
    SLOTS = so
    NT = SLOTS // 128
    slot_w = np.concatenate(slot_w_list)

    # tiles: windows overlapped; MM list (tile, window, iota_k)
    first_w = np.empty(NT, np.int64)
    mm_list = []           # (t, w, k) in emission order
    for t in range(NT):
        ws_here = slot_w[t * 128:(t + 1) * 128]
        ws_u = np.unique(ws_here[ws_here >= 0])
        fw = int(ws_u[0]) if len(ws_u) else 0
        first_w[t] = fw
        for w_ in ws_u:
            k = int(w_ - fw)
            assert 0 <= k < 5
            mm_list.append((t, int(w_), k))
    # psum-bank accumulation flags over mm_list order
    NBK = _ceil(NW, 4)
    firstmm = np.full(NBK, -1, np.int64)
    lastmm = np.full(NBK, -1, np.int64)
    for i, (t, w_, k) in enumerate(mm_list):
        bk = w_ // 4
        if firstmm[bk] < 0:
            firstmm[bk] = i
        lastmm[bk] = i
    assert (firstmm >= 0).all(), "every psum bank needs at least one MM"
    assert len({w_ for (_, w_, _k) in mm_list}) == NW, \
        "every window needs at least one MM"
    mm_flags = [(t, w_, k, i == firstmm[w_ // 4], i == lastmm[w_ // 4])
                for i, (t, w_, k) in enumerate(mm_list)]

    # per-core slot placement
    order = np.lexsort((srow, cell, core))
    core_s = core[order]
    cell_s = cell[order]
    keyall = core_s * (NW * 4) + cell_s
    starts = np.r_[0, np.flatnonzero(np.diff(keyall)) + 1]
    gid = np.zeros(len(keyall), np.int64)
    gid[starts[1:]] = 1
    gid = np.cumsum(gid)
    pos = np.arange(len(keyall)) - starts[gid]
    slot = cell_slot_off[cell_s] + pos
    assert (pos < cap[cell_s]).all()

    gidx_all = np.zeros((NCORES, SLOTS), np.int16)
    dstrel_all = np.full((NCORES, SLOTS), -1.0, np.float32)
    gidx_all[core_s, slot] = srow[order].astype(np.int16)
    # dstrel relative to the containing tile's first window
    tile_of_slot = slot // 128
    dstrel_all[core_s, slot] = (dl[order] - first_w[tile_of_slot] * 128
                                ).astype(np.float32)
    assert (dstrel_all[core_s, slot] >= 0).all()

    # device layouts
    gidx_dev = np.tile(
        gidx_all.reshape(NCORES, SLOTS // 16, 16).transpose(0, 2, 1), (1, 8, 1)
    ).copy()                                               # [8, 128, SLOTS//16]
    dstrel_dev = dstrel_all.reshape(NCORES, NT, 128).transpose(0, 2, 1).copy()

    batch = np.asarray(batch, np.int64)
    brel = np.full((NCORES, NPAD), -1.0, np.float32)
    for cc in range(NCORES):
        brel[cc, :NLOC] = batch[cc * NLOC:(cc + 1) * NLOC]
    batchrel_dev = brel.reshape(NCORES, NW, 128).transpose(0, 2, 1).copy()

    x = np.asarray(x, np.float32)
    xt_dev = np.zeros((NCORES, 128, NPAD), bf16)
    dinvT_dev = np.zeros((NCORES, 128, NPAD), np.float32)
    for cc in range(NCORES):
        xl = x[cc * NLOC:(cc + 1) * NLOC]                  # [NLOC, 128]
        xt_dev[cc, :, :NLOC] = xl.T.astype(bf16)
        dv = np.zeros(NPAD, np.float32)
        dv[:NLOC] = dinv[cc * NLOC:(cc + 1) * NLOC]
        dinvT_dev[cc] = np.broadcast_to(dv, (128, NPAD))

    meta = dict(N=N, NLOC=NLOC, NPAD=NPAD, NW=NW, NSB=NSB,
                NT=NT, SLOTS=SLOTS, groups=groups, mm_flags=mm_flags,
                CHR=CHR, NWq=NWq, qb=qb)
    data = dict(gidx=gidx_dev, dstrel=dstrel_dev, batchrel=batchrel_dev,
                xt=xt_dev, dinvt=dinvT_dev)
    return meta, data


def _epilogue(nc, sb, ps, h, dinvT, bg, l, NW, epip, AT, Relu):
    """h[:, sb windows] = relu(h + dinv*agg + bg[l])"""
    f32 = mybir.dt.float32
    ws0 = sb * SBW
    wn = min(SBW, NW - ws0)
    cs = slice(ws0 * 128, (ws0 + wn) * 128)
    u = epip.tile([128, wn * 128], f32, tag="u", bufs=2, name="u")
    nc.vector.tensor_tensor(out=u[:], in0=ps[:, :wn * 128],
                            in1=dinvT[:, cs], op=AT.mult)
    nc.vector.tensor_tensor(out=u[:], in0=u[:], in1=h[:, cs], op=AT.add)
    nc.scalar.activation(out=h[:, cs], in_=u[:], func=Relu, bias=bg[:, l:l + 1])


# --------------------------------------------------------------------------
# device program
# --------------------------------------------------------------------------
def _build(meta, L, ablate=()):
    ablate = set(ablate)
    f32 = mybir.dt.float32
    b16 = mybir.dt.bfloat16
    i16 = mybir.dt.int16
    NPAD, NW, NSB = meta["NPAD"], meta["NW"], meta["NSB"]
    NT, SLOTS = meta["NT"], meta["SLOTS"]
    groups, mm_flags = meta["groups"], meta["mm_flags"]
    CHR, NWq, qb = meta["CHR"], meta["NWq"], meta["qb"]
    SBWE = min(SBW, NW)
    rg = [list(range(NCORES))]
    # mm_flags grouped per tile for emission
    mm_by_tile = {}
    for (t, w_, k, st_f, sp_f) in mm_flags:
        mm_by_tile.setdefault(t, []).append((w_, k, st_f, sp_f))

    nc = bacc.Bacc("TRN2", target_bir_lowering=False, debug=False,
                   num_devices=NCORES)
    d_xt = nc.dram_tensor("xt", [128, NPAD], b16, kind="ExternalInput")
    d_dinvt = nc.dram_tensor("dinvt", [128, NPAD], f32, kind="ExternalInput")
    d_gidx = nc.dram_tensor("gidx", [128, SLOTS // 16], i16, kind="ExternalInput")
    d_dstrel = nc.dram_tensor("dstrel", [128, NT], f32, kind="ExternalInput")
    d_batchrel = nc.dram_tensor("batchrel", [128, NW], f32, kind="ExternalInput")
    d_w0 = nc.dram_tensor("w0", [128, 128], b16, kind="ExternalInput")
    d_wg = nc.dram_tensor("wg", [L, 128, 128], b16, kind="ExternalInput")
    d_wc1 = nc.dram_tensor("wc1", [128, 128], b16, kind="ExternalInput")
    d_wc2 = nc.dram_tensor("wc2", [128, C], b16, kind="ExternalInput")
    d_b0 = nc.dram_tensor("b0", [128, 1], f32, kind="ExternalInput")
    d_bg = nc.dram_tensor("bg", [L, 128, 1], f32, kind="ExternalInput")
    d_bc1 = nc.dram_tensor("bc1", [128, 1], f32, kind="ExternalInput")
    d_bc2m = nc.dram_tensor("bc2m", [G, C], f32, kind="ExternalInput")
    d_iota = nc.dram_tensor("iota", [128, 5 * 128], f32, kind="ExternalInput")
    d_iotag = nc.dram_tensor("iotag", [128, G], f32, kind="ExternalInput")
    d_id128 = nc.dram_tensor("id128", [128, 128], b16, kind="ExternalInput")
    d_idg = nc.dram_tensor("idg", [G, G], b16, kind="ExternalInput")
    d_out = nc.dram_tensor("out", [G, C], f32, kind="ExternalOutput")

    ag_in = [nc.dram_tensor(f"ag_in{q}", [128, NWq[q], 128], b16)
             if NWq[q] > 0 else None for q in range(4)]
    xw_q = [nc.dram_tensor(f"xw_q{q}", [CHR[q], 128], b16, addr_space="Shared")
            if NWq[q] > 0 else None for q in range(4)]
    pool_in = nc.dram_tensor("pool_in", [G, 128], f32)
    pool_out = nc.dram_tensor("pool_out", [G, 128], f32, addr_space="Shared")

    Relu = mybir.ActivationFunctionType.Relu
    Exp = mybir.ActivationFunctionType.Exp
    AT = mybir.AluOpType

    with tile.TileContext(nc) as tc:
        with (
            tc.tile_pool(name="state", bufs=1) as state,
            tc.tile_pool(name="wpool", bufs=1) as wpool,
            tc.tile_pool(name="xin", bufs=3) as xinp,
            tc.tile_pool(name="xws", bufs=3) as xwsp,
            tc.tile_pool(name="gix", bufs=2) as gixp,
            tc.tile_pool(name="gbf", bufs=3) as gbfp,
            tc.tile_pool(name="sel", bufs=3) as selp,
            tc.tile_pool(name="epi", bufs=6) as epip,
            tc.tile_pool(name="psxw", bufs=2, space="PSUM") as psxw,
            tc.tile_pool(name="pstr", bufs=2, space="PSUM") as pstr,
            tc.tile_pool(name="pswin", bufs=2, space="PSUM") as pswin,
        ):
            # ---- persistent state + constants ----
            h = state.tile([128, NPAD], b16, tag="h")
            dinvT = state.tile([128, NPAD], f32, tag="dinvT")
            dstrel = state.tile([128, NT], f32, tag="dstrel")
            xwp = state.tile([128, NW, 128], b16, tag="xwp")
            nc.sync.dma_start(dinvT[:], d_dinvt[:])
            nc.sync.dma_start(dstrel[:], d_dstrel[:])

            w0 = wpool.tile([128, 128], b16, tag="w0")
            nc.sync.dma_start(w0[:], d_w0[:])
            wg = wpool.tile([128, L, 128], b16, tag="wg")
            nc.sync.dma_start(wg[:], d_wg.rearrange("l p f -> p l f"))
            wc1 = wpool.tile([128, 128], b16, tag="wc1")
            nc.sync.dma_start(wc1[:], d_wc1[:])
            wc2 = wpool.tile([128, C], b16, tag="wc2")
            nc.sync.dma_start(wc2[:], d_wc2[:])
            b0 = wpool.tile([128, 1], f32, tag="b0")
            nc.sync.dma_start(b0[:], d_b0[:])
            bg = wpool.tile([128, L], f32, tag="bg")
            nc.sync.dma_start(bg[:], d_bg.rearrange("l p o -> p (l o)"))
            bc1 = wpool.tile([128, 1], f32, tag="bc1")
            nc.sync.dma_start(bc1[:], d_bc1[:])
            bc2m = wpool.tile([G, C], f32, tag="bc2m")
            nc.sync.dma_start(bc2m[:], d_bc2m[:])
            iota = wpool.tile([128, 5 * 128], f32, tag="iota")
            nc.sync.dma_start(iota[:], d_iota[:])
            iotag = wpool.tile([128, G], f32, tag="iotag")
            nc.sync.dma_start(iotag[:], d_iotag[:])
            id128 = wpool.tile([128, 128], b16, tag="id128")
            nc.sync.dma_start(id128[:], d_id128[:])
            idg = wpool.tile([G, G], b16, tag="idg")
            nc.sync.dma_start(idg[:], d_idg[:])
            batchrel = wpool.tile([128, NW], f32, tag="batchrel")
            nc.sync.dma_start(batchrel[:], d_batchrel[:])

            nchunks = _ceil(NPAD, 512)

            import os as _os
            for _krep in range(int(_os.environ.get("BENCH_KREP", "1"))):
                # ---- stage 1: h = relu(W0.T @ xT + b0) ----
                for k in range(nchunks):
                    c0 = k * 512
                    cw = min(512, NPAD - c0)
                    xts = xinp.tile([128, cw], b16, tag="xts")
                    nc.sync.dma_start(xts[:], d_xt[:, c0:c0 + cw])
                    ps = psxw.tile([128, cw], f32, tag="psxw")
                    nc.tensor.matmul(ps[:], lhsT=w0[:], rhs=xts[:],
                                     start=True, stop=True)
                    nc.scalar.activation(out=h[:, c0:c0 + cw], in_=ps[:],
                                         func=Relu, bias=b0[:])

                # ---- GCN layers ----
                for l in range(L):
                    # phase A: xw'T = (Wg.T @ h) * dinv  -> transpose -> ag_in
                    for k in range(nchunks):
                        c0 = k * 512
                        cw = min(512, NPAD - c0)
                        ps = psxw.tile([128, cw], f32, tag="psxw")
                        nc.tensor.matmul(ps[:], lhsT=wg[:, l, :], rhs=h[:, c0:c0 + cw],
                                         start=True, stop=True)
                        xws = xwsp.tile([128, cw], b16, tag="xws")
                        nc.vector.tensor_tensor(out=xws[:], in0=ps[:],
                                             in1=dinvT[:, c0:c0 + cw], op=AT.mult)
                        for j in range(cw // 128):
                            a = (c0 + j * 128) // 128
                            pst = pstr.tile([128, 128], b16, tag="pstr")
                            nc.tensor.transpose(pst[:], xws[:, j * 128:(j + 1) * 128],
                                                id128[:])
                            nc.vector.tensor_copy(out=xwp[:, a, :], in_=pst[:])
                    for q in range(4):
                        if NWq[q] == 0:
                            continue
                        nc.sync.dma_start(ag_in[q][:], xwp[:, qb[q]:qb[q + 1], :])
                        if "noag" in ablate:
                            nc.sync.dma_start(
                                bass.AP(xw_q[q], 0, ag_in[q][:].ap), ag_in[q][:])
                        else:
                            nc.gpsimd.collective_compute(
                                "AllGather", AT.bypass, ins=[ag_in[q][:]],
                                outs=[xw_q[q][:]], replica_groups=rg)

                    # phase B: gather + segment-sum into per-window psum
                    ps_sb = {}
                    sel_tiles = {}

                    def get_sel(t):
                        s = t // SLAB
                        if s not in sel_tiles:
                            t0 = s * SLAB
                            tn = min(SLAB, NT - t0)
                            st = selp.tile([128, tn, 128], b16, tag="sel")
                            in0 = bass.AP(dstrel.tensor,
                                          dstrel[:, t0:t0 + tn].offset,
                                          [dstrel[:].ap[0], [1, tn], [0, 128]])
                            in1 = bass.AP(iota.tensor, iota[:].offset,
                                          [iota[:].ap[0], [0, tn], [1, 128]])
                            nc.vector.tensor_tensor(out=st[:], in0=in0, in1=in1,
                                                    op=AT.is_equal)
                            sel_tiles.clear()
                            sel_tiles[s] = st
                        return sel_tiles[s], t - s * SLAB

                    for (sb, c4, g_off, g_nt) in groups:
                        if sb not in ps_sb:
                            ps_sb.clear()
                            ps_sb[sb] = pswin.tile([128, SBWE * 128], f32,
                                                   name="pswin_t", tag="pswin")
                        if g_nt == 0:
                            if c4 == 3:
                                _epilogue(nc, sb, ps_sb[sb], h, dinvT, bg, l, NW,
                                          epip, AT, Relu)
                            continue
                        slots = g_nt * 128
                        gb = gbfp.tile([128, g_nt, 128], b16, tag="gbf")
                        gixt = gixp.tile([128, slots // 16], i16, tag="gix")
                        so = g_off * 128
                        nc.sync.dma_start(gixt[:],
                                          d_gidx[:, so // 16:(so + slots) // 16])
                        nc.gpsimd.dma_gather(
                            gb[:], xw_q[c4][:], gixt[:], slots, slots, 128,
                            single_packet=False)
                        for ti in range(g_nt):
                            t = g_off + ti
                            st, si = get_sel(t)
                            for (w_, k, st_f, sp_f) in mm_by_tile.get(t, []):
                                wr = w_ - sb * SBW
                                if k == 0:
                                    rhs = st[:, si, :]
                                else:
                                    sk = selp.tile([128, 1, 128], b16, name="sk",
                                                   tag="selk", bufs=2)
                                    in0 = bass.AP(
                                        dstrel.tensor, dstrel[:, t:t + 1].offset,
                                        [dstrel[:].ap[0], [1, 1], [0, 128]])
                                    in1 = bass.AP(
                                        iota.tensor, iota[:, k * 128:].offset,
                                        [iota[:].ap[0], [0, 1], [1, 128]])
                                    nc.vector.tensor_tensor(
                                        out=sk[:], in0=in0, in1=in1, op=AT.is_equal)
                                    rhs = sk[:, 0, :]
                                nc.tensor.matmul(
                                    ps_sb[sb][:, wr * 128:(wr + 1) * 128],
                                    lhsT=gb[:, ti, :], rhs=rhs,
                                    start=bool(st_f), stop=bool(sp_f))
                        if c4 == 3:
                            _epilogue(nc, sb, ps_sb[sb], h, dinvT, bg, l, NW,
                                      epip, AT, Relu)

                # ---- global add pool ----
                psp = psxw.tile([G, 128], f32, tag="psxw")
                bsel = None
                for a in range(NW):
                    pst = pstr.tile([128, 128], b16, tag="pstr")
                    nc.tensor.transpose(pst[:], h[:, a * 128:(a + 1) * 128], id128[:])
                    hn = epip.tile([128, 128], b16, tag="hn")
                    nc.vector.tensor_copy(out=hn[:], in_=pst[:])
                    if a % SLAB == 0:
                        a0 = a
                        an = min(SLAB, NW - a0)
                        bsel = selp.tile([128, an, G], b16, tag="sel")
                        in0 = bass.AP(batchrel.tensor, batchrel[:, a0:a0 + an].offset,
                                      [batchrel[:].ap[0], [1, an], [0, G]])
                        in1 = bass.AP(iotag.tensor, iotag[:].offset,
                                      [iotag[:].ap[0], [0, an], [1, G]])
                        nc.vector.tensor_tensor(out=bsel[:], in0=in0, in1=in1,
                                             op=AT.is_equal)
                    nc.tensor.matmul(psp[:], lhsT=bsel[:, a - a0, :], rhs=hn[:],
                                     start=(a == 0), stop=(a == NW - 1))
                pool_sb = epip.tile([G, 128], f32, tag="poolsb")
                nc.vector.tensor_copy(out=pool_sb[:], in_=psp[:])
                nc.sync.dma_start(pool_in[:], pool_sb[:])
                nc.gpsimd.collective_compute(
                    "AllReduce", AT.add, ins=[pool_in[:]], outs=[pool_out[:]],
                    replica_groups=rg)

                # ---- classifier (replicated) ----
                pooled_f = epip.tile([G, 128], f32, tag="pooledf")
                nc.sync.dma_start(pooled_f[:], pool_out[:])
                pooled_b = epip.tile([G, 128], b16, tag="pooledb")
                nc.vector.tensor_copy(out=pooled_b[:], in_=pooled_f[:])
                pstp = pstr.tile([128, G], b16, tag="pstr")
                nc.tensor.transpose(pstp[:], pooled_b[:], idg[:])
                pooledT = epip.tile([128, G], b16, tag="pooledT")
                nc.vector.tensor_copy(out=pooledT[:], in_=pstp[:])
                psz = pstr.tile([128, G], f32, tag="pstr")
                nc.tensor.matmul(psz[:], lhsT=wc1[:], rhs=pooledT[:],
                                 start=True, stop=True)
                zt = epip.tile([128, G], b16, tag="zt")
                nc.scalar.activation(out=zt[:], in_=psz[:], func=Relu, bias=bc1[:])
                pslg = pstr.tile([G, C], f32, tag="pstr")
                nc.tensor.matmul(pslg[:], lhsT=zt[:], rhs=wc2[:],
                                 start=True, stop=True)
                lg = epip.tile([G, C], f32, tag="lg")
                nc.vector.tensor_tensor(out=lg[:], in0=pslg[:], in1=bc2m[:], op=AT.add)
                # softmax over C (free dim)
                mx = epip.tile([G, 1], f32, tag="mx")
                nc.vector.tensor_reduce(out=mx[:], in_=lg[:],
                                        axis=mybir.AxisListType.X, op=AT.max)
                nmx = epip.tile([G, 1], f32, tag="nmx")
                nc.vector.tensor_scalar_mul(nmx[:], mx[:], -1.0)
                ex = epip.tile([G, C], f32, tag="ex")
                nc.scalar.activation(out=ex[:], in_=lg[:], func=Exp, bias=nmx[:])
                sm = epip.tile([G, 1], f32, tag="sm")
                nc.vector.tensor_reduce(out=sm[:], in_=ex[:],
                                        axis=mybir.AxisListType.X, op=AT.add)
                rs = epip.tile([G, 1], f32, tag="rs")
                nc.vector.reciprocal(rs[:], sm[:])
                prob = epip.tile([G, C], f32, tag="prob")
                nc.vector.tensor_scalar_mul(prob[:], ex[:], rs[:])
                nc.sync.dma_start(d_out[:], prob[:])

    nc.compile()
    return nc


# --------------------------------------------------------------------------
# entry point
# --------------------------------------------------------------------------
def kernel(x, edge_index, batch, W0, b0, Wg, bg, Wc1, bc1, Wc2, bc2,
           **extra):
    x = np.asarray(x, np.float32)
    edge_index = np.asarray(edge_index)
    batch = np.asarray(batch)
    W0 = np.asarray(W0, np.float32)
    Wg = np.asarray(Wg, np.float32)
    L = Wg.shape[0]

    key = (x.shape, edge_index.shape,
           hash(edge_index.tobytes()), hash(np.asarray(batch).tobytes()))
    if key not in _cache:
        meta, data = _preprocess(x, edge_index, batch)
        nc = _build(meta, L)
        _cache.clear()
        _cache[key] = (meta, data, nc)
    meta, data, nc = _cache[key]

    iota = np.broadcast_to(np.arange(5 * 128, dtype=np.float32),
                           (128, 5 * 128)).copy()
    iotag = np.broadcast_to(np.arange(G, dtype=np.float32), (128, G)).copy()
    common = dict(
        w0=W0.astype(bf16).view(np.uint16),
        wg=Wg.astype(bf16).view(np.uint16),
        wc1=np.asarray(Wc1, np.float32).astype(bf16).view(np.uint16),
        wc2=np.asarray(Wc2, np.float32).astype(bf16).view(np.uint16),
        b0=np.asarray(b0, np.float32).reshape(128, 1),
        bg=np.asarray(bg, np.float32).reshape(L, 128, 1),
        bc1=np.asarray(bc1, np.float32).reshape(128, 1),
        bc2m=np.broadcast_to(np.asarray(bc2, np.float32), (G, C)).copy(),
        iota=iota, iotag=iotag,
        id128=np.eye(128, dtype=np.float32).astype(bf16).view(np.uint16),
        idg=np.eye(G, dtype=np.float32).astype(bf16).view(np.uint16),
    )
    in_maps = []
    for c in range(NCORES):
        m = dict(common)
        m["xt"] = data["xt"][c].view(np.uint16)
        m["dinvt"] = data["dinvt"][c]
        m["gidx"] = data["gidx"][c]
        m["dstrel"] = data["dstrel"][c]
        m["batchrel"] = data["batchrel"][c]
        in_maps.append(m)

    import os
    trace = os.environ.get("BASS_KERNEL_TRACE", "0") == "1"
    res = run_bass_kernel_spmd(nc, in_maps, list(range(NCORES)), trace=trace)
    kernel._last_exec_ns = res.exec_time_ns
    kernel._last_results = res
    return np.asarray(res.results[0]["out"], np.float32)


kernel._last_exec_ns = None



# revision 35
# speedup vs baseline: 3.6364x; 3.6364x over previous
"""3-layer GCN (GCNConv + residual + relu, global add pool, MLP softmax) on 8
Trainium2 NeuronCores.

Sharding: nodes/edges partitioned by destination across the 8 cores, with a
host-side LPT rebalance (node relabeling) that equalizes per-(window, chunk)
edge counts across cores to minimize SPMD slot padding. Per layer: each core
computes its shard of the message table xw' = (Wg.T h) * dinv_src, AllGathers
it (bf16, in 4 overlapping quarter chunks, staggered so each quarter's
AllGather overlaps the previous layer's gathers; tables double-buffered by
layer parity), then gathers per-edge rows with dma_gather and segment-sums
them into per-64-dst-window PSUM accumulators via one-hot matmuls (sel built
on-device comparing fp16 dstrel against an iota row). The host balancer
levels every (window, chunk) cell to exactly 256 slots, so tiles never span
windows: one 64-wide MM per gathered tile. Self-loop terms are computed
locally in the epilogue (never gathered); the next layer's xw'=h@Wg chunks
and the global-add-pool partials are emitted inside the epilogues for
cross-phase overlap. All cores run the IDENTICAL program; per-core variation
lives entirely in data (gather indices, sel values, padding). Pooled
[64,128] partials are AllReduced; the tiny classifier is replicated.
"""
import os
import numpy as np
import ml_dtypes

import concourse.bacc as bacc
import concourse.bass as bass
import concourse.mybir as mybir
import concourse.tile as tile
from concourse.bass_utils import run_bass_kernel_spmd

NCORES = 8
G = 64     # graphs in batch
C = 2      # classes
W = 64     # dst window width (one-hot / psum granularity)
SBWIN = 16  # dst windows per superblock (16*64 = 1024 dst = 2 psum banks)
SLAB = 32  # sel tiles built per is_equal op

NPAD = 12544          # padded nodes per core
NWIN = NPAD // W      # 196 dst windows per core
NW = NPAD // 128      # 98 "a-tiles" (128-node blocks) per core
NSB = -(-NWIN // SBWIN)  # 13 superblocks
# gather-table quarters: CYCLIC overlapping a-tile ranges (32 tiles each,
# starts spaced ~24.5) so every quarter overlaps both neighbours. Edges whose
# source a-tile lies in an overlap may use either chunk, giving the host
# balancer freedom to level per-(window, chunk) counts to a 256-slot cap
# (= exactly 2 gather tiles, so cell boundaries stay tile-aligned).
QS = [0, 25, 49, 74]       # cyclic start tile of each quarter
NWQ = 30                   # a-tiles per quarter
CHR = NCORES * 128 * NWQ   # 30720 rows per chunk table
CAPT = 256                 # target slot cap per (window, chunk) cell


def _qtiles(q):
    """a-tile list of quarter q, in srow order."""
    return [(QS[q] + i) % NW for i in range(NWQ)]

bf16 = ml_dtypes.bfloat16
_cache = {}


def _ceil(a, b):
    return -(-a // b)


# --------------------------------------------------------------------------
# host preprocessing
# --------------------------------------------------------------------------
def _balance_nodes(indeg):
    """LPT: assign nodes to NCORES*NWIN global windows of W slots, balancing
    the gathered in-degree sum per window. Returns (core, loc) per node."""
    import heapq
    N = len(indeg)
    GW = NCORES * NWIN
    assert GW * W >= N
    order = np.argsort(-indeg, kind="stable")
    heap = [(0, w) for w in range(GW)]
    heapq.heapify(heap)
    room = np.full(GW, W, np.int64)
    fill = np.zeros(GW, np.int64)
    gwin = np.empty(N, np.int64)
    pos = np.empty(N, np.int64)
    for nd in order:
        while True:
            load, wbin = heapq.heappop(heap)
            if room[wbin] > 0:
                break
        gwin[nd] = wbin
        pos[nd] = fill[wbin]
        fill[wbin] += 1
        room[wbin] -= 1
        if room[wbin] > 0:
            heapq.heappush(heap, (load + int(indeg[nd]), wbin))
    core = gwin % NCORES
    windex = gwin // NCORES
    loc = windex * W + pos
    return core, loc


def _preprocess(x, edge_index, batch):
    N, D = x.shape
    assert D == 128

    src = np.asarray(edge_index[0], np.int64)
    dst = np.asarray(edge_index[1], np.int64)
    E = len(src)
    deg = np.bincount(dst, minlength=N).astype(np.float64) + 1.0
    dinv = (deg ** -0.5).astype(np.float32)
    indeg = np.bincount(dst, minlength=N)  # gathered (non-loop) in-degree

    core_of, loc_of = _balance_nodes(indeg)

    # --- per-core edge slot layout ------------------------------------
    csrc, lsrc = core_of[src], loc_of[src]
    cdst, ldst = core_of[dst], loc_of[dst]
    p_s = lsrc % 128
    a_s = lsrc // 128
    w64 = ldst // W

    # chunk assignment: a-tile t belongs to quarter q iff (t-QS[q]) mod 98
    # < 32. Forced when only one quarter covers it; flexible (overlap of
    # quarters p and p+1 mod 4) edges are split per (core, dst-window) to
    # level the four chunk counts to <= CAPT.
    memb = np.stack([(a_s - QS[q]) % NW < NWQ for q in range(4)])  # [4, E]
    nmemb = memb.sum(axis=0)
    assert ((nmemb == 1) | (nmemb == 2)).all()
    ch = np.full(E, -1, np.int64)
    flex = np.full(E, -1, np.int64)   # pool id p: chunk p or (p+1)%4
    for q in range(4):
        only = memb[q] & (nmemb == 1)
        ch[only] = q
        both = memb[q] & memb[(q + 1) % 4]
        flex[both] = q
    assert ((ch >= 0) ^ (flex >= 0)).all()

    # per (core, window, chunk) forced counts and flexible pool counts
    fcnt = np.zeros((NCORES, NWIN, 4), np.int64)
    fmask = ch >= 0
    np.add.at(fcnt, (cdst[fmask], w64[fmask], ch[fmask]), 1)
    xcnt = np.zeros((NCORES, NWIN, 4), np.int64)
    xmask = flex >= 0
    np.add.at(xcnt, (cdst[xmask], w64[xmask], flex[xmask]), 1)

    # cyclic balance: take_left[p] = # of pool-p edges assigned to chunk p
    # (rest go to chunk (p+1)%4). Equalizing sweeps, then overflow-draining
    # passes pushing any chunk above CAPT down through its pools.
    take_left = xcnt.copy()  # start: all pool edges to left chunk
    ccnt = fcnt.copy()
    for p in range(4):
        ccnt[:, :, p] += take_left[:, :, p]
    for it in range(24):
        for p in range(4):
            a = ccnt[:, :, p]
            b = ccnt[:, :, (p + 1) % 4]
            if it < 8:
                shift = (a - b) // 2  # >0: move left->right
            else:
                # drain overflow only
                shift = (np.maximum(a - CAPT, 0)
                         - np.maximum(b - CAPT, 0))
            shift = np.clip(shift, take_left[:, :, p] - xcnt[:, :, p],
                            take_left[:, :, p])
            take_left[:, :, p] -= shift
            ccnt[:, :, p] -= shift
            ccnt[:, :, (p + 1) % 4] += shift

    # resolve flexible edges: first take_left (by order) -> chunk p, rest p+1
    if xmask.any():
        xi = np.flatnonzero(xmask)
        okey = cdst[xi] * (NWIN * 4) + w64[xi] * 4 + flex[xi]
        oorder = np.argsort(okey, kind="stable")
        xi = xi[oorder]
        okey = okey[oorder]
        ost = np.r_[0, np.flatnonzero(np.diff(okey)) + 1]
        og = np.zeros(len(okey), np.int64)
        og[ost[1:]] = 1
        og = np.cumsum(og)
        opos = np.arange(len(okey)) - ost[og]
        tl = take_left.reshape(-1)[okey]
        ch[xi] = np.where(opos < tl, flex[xi], (flex[xi] + 1) % 4)
    assert (ch >= 0).all()

    qs_arr = np.asarray(QS, np.int64)
    srow = (csrc * (128 * NWQ) + p_s * NWQ + (a_s - qs_arr[ch]) % NW)
    assert srow.min() >= 0 and srow.max() < 32768

    cell = w64 * 4 + ch
    key = cdst * (NWIN * 4) + cell
    counts = np.bincount(key, minlength=NCORES * NWIN * 4)
    counts = counts.reshape(NCORES, NWIN * 4)
    cap = _ceil(np.maximum(counts.max(axis=0), 1), 128) * 128
    novf = int((cap > CAPT).sum())
    if novf > 40:
        import sys
        print(f"[kernel] warning: {novf} cells over CAPT", file=sys.stderr)

    # stream layout: sb -> ch -> w; groups padded to x128
    cell_slot_off = np.zeros(NWIN * 4, np.int64)
    groups = []           # (sb, ch, tile_off, n_tiles)
    slot_w_list = []      # per-slot window id, -1 = pad
    so = 0
    for sb in range(NSB):
        ws = range(sb * SBWIN, min((sb + 1) * SBWIN, NWIN))
        for c4 in range(4):
            g_so = so
            for w_ in ws:
                cid = w_ * 4 + c4
                cell_slot_off[cid] = so
                slot_w_list.append(np.full(int(cap[cid]), w_, np.int64))
                so += int(cap[cid])
            pad = (-(so - g_so)) % 128
            if pad:
                slot_w_list.append(np.full(pad, -1, np.int64))
                so += pad
            groups.append((sb, c4, g_so // 128, (so - g_so) // 128))
    SLOTS = so
    NT = SLOTS // 128
    slot_w = np.concatenate(slot_w_list)

    # static per-tile structure: crossing-free -> exactly one MM per tile
    first_w = np.zeros(NT, np.int64)
    mm_by_group = {}      # (sb,c4) -> [(t, rel)]
    for (sb, c4, g_off, g_nt) in groups:
        mms = []
        for ti in range(g_nt):
            t = g_off + ti
            ws_here = slot_w[t * 128:(t + 1) * 128]
            ws_u = np.unique(ws_here[ws_here >= 0])
            if len(ws_u) == 0:
                continue
            assert len(ws_u) == 1, "tile spans >1 window (caps not x128?)"
            w0 = int(ws_u[0])
            first_w[t] = w0
            mms.append((t, w0 - sb * SBWIN))
        mm_by_group[(sb, c4)] = mms

    # psum accumulation flags per (sb, bank): first/last in emission order
    mm_flags = {}         # (sb,c4) -> [(t, rel, start, stop)]
    for sb in range(NSB):
        seq = []
        for c4 in range(4):
            for m in mm_by_group[(sb, c4)]:
                seq.append((c4, m))
        firstmm = {}
        lastmm = {}
        for i, (c4, (t, rel)) in enumerate(seq):
            bk = rel // 8
            if bk not in firstmm:
                firstmm[bk] = i
            lastmm[bk] = i
        nwin_sb = min(SBWIN, NWIN - sb * SBWIN)
        for bk in range(_ceil(nwin_sb, 8)):
            assert bk in firstmm, f"psum bank {sb}/{bk} has no MM"
        for c4 in range(4):
            mm_flags[(sb, c4)] = []
        for i, (c4, (t, rel)) in enumerate(seq):
            bk = rel // 8
            mm_flags[(sb, c4)].append(
                (t, rel, i == firstmm[bk], i == lastmm[bk]))

    # --- per-core slot placement --------------------------------------
    order = np.lexsort((srow, cell, cdst))
    core_s = cdst[order]
    cell_s = cell[order]
    keyall = core_s * (NWIN * 4) + cell_s
    starts = np.r_[0, np.flatnonzero(np.diff(keyall)) + 1]
    gid = np.zeros(len(keyall), np.int64)
    gid[starts[1:]] = 1
    gid = np.cumsum(gid)
    pos = np.arange(len(keyall)) - starts[gid]
    slot = cell_slot_off[cell_s] + pos
    assert (pos < cap[cell_s]).all()

    gidx_all = np.zeros((NCORES, SLOTS), np.int16)
    dstrel_all = np.full((NCORES, SLOTS), -1.0, np.float32)
    gidx_all[core_s, slot] = srow[order].astype(np.int16)
    tile_of_slot = slot // 128
    dstrel_all[core_s, slot] = (ldst[order] - first_w[tile_of_slot] * W
                                ).astype(np.float32)
    dr = dstrel_all[core_s, slot]
    assert (dr >= 0).all() and (dr < W).all()

    # --- device layouts -----------------------------------------------
    gidx_dev = np.tile(
        gidx_all.reshape(NCORES, SLOTS // 16, 16).transpose(0, 2, 1), (1, 8, 1)
    ).copy()                                           # [8, 128, SLOTS//16]
    dstrel_dev = dstrel_all.reshape(NCORES, NT, 128).transpose(0, 2, 1) \
        .astype(np.float16).copy()                     # [8, 128, NT]

    batch = np.asarray(batch, np.int64)
    brel = np.full((NCORES, NPAD), -1.0, np.float32)
    brel[core_of, loc_of] = batch.astype(np.float32)
    batchrel_dev = brel.reshape(NCORES, NW, 128).transpose(0, 2, 1).copy()

    x = np.asarray(x, np.float32)
    xt_dev = np.zeros((NCORES, 128, NPAD), bf16)
    xt_dev[core_of, :, loc_of] = x.astype(bf16)
    dinvT_dev = np.zeros((NCORES, 128, NPAD), bf16)
    dv = np.zeros((NCORES, NPAD), np.float32)
    dv[core_of, loc_of] = dinv
    for cc in range(NCORES):
        dinvT_dev[cc] = np.broadcast_to(dv[cc], (128, NPAD)).astype(bf16)

    meta = dict(N=N, NT=NT, SLOTS=SLOTS, groups=groups, mm_flags=mm_flags)
    data = dict(gidx=gidx_dev, dstrel=dstrel_dev, batchrel=batchrel_dev,
                xt=xt_dev, dinvt=dinvT_dev)
    return meta, data


# --------------------------------------------------------------------------
# device program
# --------------------------------------------------------------------------
def _build(meta, L, ablate=()):
    ablate = set(ablate)
    f32 = mybir.dt.float32
    f16 = mybir.dt.float16
    b16 = mybir.dt.bfloat16
    i16 = mybir.dt.int16
    NT, SLOTS = meta["NT"], meta["SLOTS"]
    groups, mm_flags = meta["groups"], meta["mm_flags"]
    rg = [list(range(NCORES))]
    # NOTE: is_equal is NOT supported on the Pool engine by the real ISA
    # (walrus rejects it) -- sel slabs must stay on DVE.
    pool_every = int(os.environ.get("SEL_POOL_EVERY", "0"))
    epi_pool = os.environ.get("EPI_POOL", "1") == "1"

    nc = bacc.Bacc("TRN2", target_bir_lowering=False, debug=False,
                   num_devices=NCORES)
    d_xt = nc.dram_tensor("xt", [128, NPAD], b16, kind="ExternalInput")
    d_dinvt = nc.dram_tensor("dinvt", [128, NPAD], b16, kind="ExternalInput")
    d_gidx = nc.dram_tensor("gidx", [128, SLOTS // 16], i16, kind="ExternalInput")
    d_dstrel = nc.dram_tensor("dstrel", [128, NT], f16, kind="ExternalInput")
    d_batchrel = nc.dram_tensor("batchrel", [128, NW], f32, kind="ExternalInput")
    d_w0 = nc.dram_tensor("w0", [128, 128], b16, kind="ExternalInput")
    d_wg = nc.dram_tensor("wg", [L, 128, 128], b16, kind="ExternalInput")
    d_wc1 = nc.dram_tensor("wc1", [128, 128], b16, kind="ExternalInput")
    d_wc2 = nc.dram_tensor("wc2", [128, C], b16, kind="ExternalInput")
    d_b0 = nc.dram_tensor("b0", [128, 1], f32, kind="ExternalInput")
    d_bg = nc.dram_tensor("bg", [L, 128, 1], f32, kind="ExternalInput")
    d_bc1 = nc.dram_tensor("bc1", [128, 1], f32, kind="ExternalInput")
    d_bc2m = nc.dram_tensor("bc2m", [G, C], f32, kind="ExternalInput")
    d_iota16 = nc.dram_tensor("iota16", [128, 128], f16, kind="ExternalInput")
    d_iotag = nc.dram_tensor("iotag", [128, G], f32, kind="ExternalInput")
    d_id128 = nc.dram_tensor("id128", [128, 128], b16, kind="ExternalInput")
    d_idg = nc.dram_tensor("idg", [G, G], b16, kind="ExternalInput")
    d_out = nc.dram_tensor("out", [G, C], f32, kind="ExternalOutput")

    ag_in = [nc.dram_tensor(f"ag_in{q}", [128, NWQ, 128], b16)
             for q in range(4)]
    # double-buffered by layer parity so next layer's AllGather overlaps
    # this layer's gathers instead of WAR-serializing on the table
    xw_q = [[nc.dram_tensor(f"xw_q{b}_{q}", [CHR, 128], b16,
                            addr_space="Shared")
             for q in range(4)] for b in range(2)]
    pool_in = nc.dram_tensor("pool_in", [G, 128], f32)
    pool_out = nc.dram_tensor("pool_out", [G, 128], f32, addr_space="Shared")

    Relu = mybir.ActivationFunctionType.Relu
    Exp = mybir.ActivationFunctionType.Exp
    Copy = mybir.ActivationFunctionType.Copy
    AT = mybir.AluOpType

    with tile.TileContext(nc) as tc:
        with (
            tc.tile_pool(name="state", bufs=1) as state,
            tc.tile_pool(name="wpool", bufs=1) as wpool,
            tc.tile_pool(name="xin", bufs=3) as xinp,
            tc.tile_pool(name="gbf", bufs=4) as gbfp,
            tc.tile_pool(name="sel", bufs=5) as selp,
            tc.tile_pool(name="epi", bufs=4) as epip,
            tc.tile_pool(name="cls", bufs=2) as clsp,
            tc.tile_pool(name="psxw", bufs=2, space="PSUM") as psxw,
            tc.tile_pool(name="pstr", bufs=2, space="PSUM") as pstr,
            tc.tile_pool(name="pswin", bufs=2, space="PSUM") as pswin,
        ):
            # ---- persistent state + constants ----
            h = state.tile([128, NPAD], b16, tag="h")
            xws = state.tile([128, NPAD], b16, tag="xws")
            xwp = state.tile([128, NW, 128], b16, tag="xwp")
            dinvT = state.tile([128, NPAD], b16, tag="dinvT")
            dstrel = state.tile([128, NT], f16, tag="dstrel")
            gidxS = state.tile([128, SLOTS // 16], i16, tag="gidxS")
            nc.sync.dma_start(dinvT[:], d_dinvt[:])
            nc.sync.dma_start(dstrel[:], d_dstrel[:])
            nc.sync.dma_start(gidxS[:], d_gidx[:])

            w0 = wpool.tile([128, 128], b16, tag="w0")
            nc.sync.dma_start(w0[:], d_w0[:])
            wg = wpool.tile([128, L, 128], b16, tag="wg")
            nc.sync.dma_start(wg[:], d_wg.rearrange("l p f -> p l f"))
            wc1 = wpool.tile([128, 128], b16, tag="wc1")
            nc.sync.dma_start(wc1[:], d_wc1[:])
            wc2 = wpool.tile([128, C], b16, tag="wc2")
            nc.sync.dma_start(wc2[:], d_wc2[:])
            b0 = wpool.tile([128, 1], f32, tag="b0")
            nc.sync.dma_start(b0[:], d_b0[:])
            bg = wpool.tile([128, L], f32, tag="bg")
            nc.sync.dma_start(bg[:], d_bg.rearrange("l p o -> p (l o)"))
            bc1 = wpool.tile([128, 1], f32, tag="bc1")
            nc.sync.dma_start(bc1[:], d_bc1[:])
            bc2m = wpool.tile([G, C], f32, tag="bc2m")
            nc.sync.dma_start(bc2m[:], d_bc2m[:])
            iota16 = wpool.tile([128, 128], f16, tag="iota16")
            nc.sync.dma_start(iota16[:], d_iota16[:])
            iotag = wpool.tile([128, G], f32, tag="iotag")
            nc.sync.dma_start(iotag[:], d_iotag[:])
            id128 = wpool.tile([128, 128], b16, tag="id128")
            nc.sync.dma_start(id128[:], d_id128[:])
            idg = wpool.tile([G, G], b16, tag="idg")
            nc.sync.dma_start(idg[:], d_idg[:])
            batchrel = wpool.tile([128, NW], f32, tag="batchrel")
            nc.sync.dma_start(batchrel[:], d_batchrel[:])

            nchunks = _ceil(NPAD, 512)
            sel_ctr = [0]
            pool_state = {}
            # quarter q's a-tiles are all transposed once the epilogue of
            # this superblock (resp. stage-1 chunk) has run
            _AGQ_AT_SB = {3: [0], 6: [1], 10: [2], 12: [3]}
            _AGQ_AT_K = {7: [0], 13: [1], 20: [2], 24: [3]}

            def emit_phase_a_chunk(l, k):
                """xws/xwp for h cols [512k, ...) using layer-l weights."""
                c0 = k * 512
                cw = min(512, NPAD - c0)
                ps = psxw.tile([128, cw], f32, tag="psxw", name="psA")
                nc.tensor.matmul(ps[:], lhsT=wg[:, l, :],
                                 rhs=h[:, c0:c0 + cw], start=True, stop=True)
                nc.vector.tensor_tensor(out=xws[:, c0:c0 + cw], in0=ps[:],
                                        in1=dinvT[:, c0:c0 + cw], op=AT.mult)
                for j in range(cw // 128):
                    a = (c0 + j * 128) // 128
                    pst = pstr.tile([128, 128], b16, tag="pstr", name="pstA")
                    nc.tensor.transpose(
                        pst[:], xws[:, c0 + j * 128:c0 + (j + 1) * 128],
                        id128[:])
                    nc.scalar.copy(out=xwp[:, a, :], in_=pst[:])

            def emit_ag(q, buf):
                s_ = QS[q]
                n1 = min(NWQ, NW - s_)
                nc.sync.dma_start(ag_in[q][:, 0:n1, :],
                                  xwp[:, s_:s_ + n1, :])
                if n1 < NWQ:
                    nc.sync.dma_start(ag_in[q][:, n1:NWQ, :],
                                      xwp[:, 0:NWQ - n1, :])
                if "noag" in ablate:
                    # model the collective's local-DMA cost: every core
                    # writes all 8 ranks' shards into its table copy
                    sz = 128 * NWQ * 128
                    for r in range(NCORES):
                        nc.sync.dma_start(
                            bass.AP(xw_q[buf][q], r * sz, ag_in[q][:].ap),
                            ag_in[q][:])
                else:
                    nc.gpsimd.collective_compute(
                        "AllGather", AT.bypass, ins=[ag_in[q][:]],
                        outs=[xw_q[buf][q][:]], replica_groups=rg)

            def emit_pool_sb(sb):
                """accumulate global-add-pool partials for sb's a-tiles."""
                if "psp" not in pool_state:
                    pool_state["psp"] = psxw.tile([G, 128], f32, tag="psxw",
                                                  name="psp")
                psp = pool_state["psp"]
                a_lo = sb * (SBWIN * W // 128)
                a_hi = min(a_lo + SBWIN * W // 128, NW)
                an = a_hi - a_lo
                bsel = selp.tile([128, an, G], b16, tag="sel", name="bsel")
                in0 = bass.AP(batchrel.tensor, batchrel[:, a_lo:a_hi].offset,
                              [batchrel[:].ap[0], [1, an], [0, G]])
                in1 = bass.AP(iotag.tensor, iotag[:].offset,
                              [iotag[:].ap[0], [0, an], [1, G]])
                nc.vector.tensor_tensor(out=bsel[:], in0=in0, in1=in1,
                                        op=AT.is_equal)
                for a in range(a_lo, a_hi):
                    pst = pstr.tile([128, 128], b16, tag="pstr", name="pstP")
                    nc.tensor.transpose(pst[:], h[:, a * 128:(a + 1) * 128],
                                        id128[:])
                    hn = clsp.tile([128, 128], b16, tag="hn", name="hn")
                    nc.scalar.copy(out=hn[:], in_=pst[:])
                    nc.tensor.matmul(psp[:], lhsT=bsel[:, a - a_lo, :],
                                     rhs=hn[:],
                                     start=(a == 0), stop=(a == NW - 1))

            def emit_phase_b(l):
                """gather + one-hot matmul segment-sum + epilogue, per sb."""
                ps_sb = {}
                for (sb, c4, g_off, g_nt) in groups:
                    if sb not in ps_sb:
                        ps_sb.clear()
                        ps_sb[sb] = pswin.tile([128, SBWIN * W], f32,
                                               name="pswin_t", tag="pswin")
                    ps = ps_sb[sb]
                    if g_nt > 0:
                        slots = g_nt * 128
                        gb = gbfp.tile([128, g_nt, 128], b16, tag="gbf",
                                       name="gb")
                        so = g_off * 128
                        nc.gpsimd.dma_gather(
                            gb[:], xw_q[l % 2][c4][:],
                            gidxS[:, so // 16:(so + slots) // 16],
                            slots, slots, 128, single_packet=False)
                        # sel slabs + MMs
                        mms = mm_flags[(sb, c4)]
                        mm_by_t = {}
                        for m in mms:
                            mm_by_t.setdefault(m[0], []).append(m)
                        for s0 in range(0, g_nt, SLAB):
                            tn = min(SLAB, g_nt - s0)
                            t0 = g_off + s0
                            eng = (nc.gpsimd if pool_every > 0 and
                                   sel_ctr[0] % pool_every == pool_every - 1
                                   else nc.vector)
                            sel_ctr[0] += 1
                            st = selp.tile([128, tn, W], b16, tag="sel",
                                           name="st")
                            in0 = bass.AP(
                                dstrel.tensor, dstrel[:, t0:t0 + tn].offset,
                                [dstrel[:].ap[0], [1, tn], [0, W]])
                            in1 = bass.AP(
                                iota16.tensor, iota16[:].offset,
                                [iota16[:].ap[0], [0, tn], [1, W]])
                            eng.tensor_tensor(out=st[:], in0=in0, in1=in1,
                                              op=AT.is_equal)
                            for ti in range(s0, s0 + tn):
                                t = g_off + ti
                                si = ti - s0
                                for (t_, rel, st_f, sp_f) in \
                                        mm_by_t.get(t, []):
                                    nc.tensor.matmul(
                                        ps[:, rel * W:(rel + 1) * W],
                                        lhsT=gb[:, ti, :],
                                        rhs=st[:, si, :],
                                        start=bool(st_f), stop=bool(sp_f))
                    if c4 == 3:
                        # epilogue: h[:,sb] = relu(h + dinv*(ps + xws) + bg)
                        nwin_sb = min(SBWIN, NWIN - sb * SBWIN)
                        cs = slice(sb * SBWIN * W, sb * SBWIN * W + nwin_sb * W)
                        u = epip.tile([128, SBWIN * W], f32, tag="u", name="u")
                        un = nwin_sb * W
                        nc.vector.tensor_tensor(out=u[:, :un],
                                                in0=ps[:, :un],
                                                in1=xws[:, cs], op=AT.add)
                        nc.vector.tensor_tensor(out=u[:, :un], in0=u[:, :un],
                                                in1=dinvT[:, cs], op=AT.mult)
                        # Pool engine can't touch PSUM; this add is all-SBUF
                        eng0 = nc.gpsimd if epi_pool else nc.vector
                        eng0.tensor_tensor(out=u[:, :un], in0=u[:, :un],
                                           in1=h[:, cs], op=AT.add)
                        nc.scalar.activation(out=h[:, cs], in_=u[:, :un],
                                             func=Relu, bias=bg[:, l:l + 1])
                        # h[:, sb] is final for this layer: immediately emit
                        # the next layer's phase A for these columns (or the
                        # pooling partials after the last layer), plus any
                        # AllGather quarter whose a-tiles just completed.
                        if l < L - 1:
                            for k in range(2 * sb,
                                           min(2 * sb + 2, nchunks)):
                                emit_phase_a_chunk(l + 1, k)
                            for q in _AGQ_AT_SB.get(sb, []):
                                emit_ag(q, (l + 1) % 2)
                        else:
                            emit_pool_sb(sb)

            import os as _os
            for _krep in range(int(_os.environ.get("BENCH_KREP", "1"))):
                # ---- stage 1 (h = relu(W0.T xT + b0)) fused with layer-0
                # phase A ----
                pool_state.clear()
                for k in range(nchunks):
                    c0 = k * 512
                    cw = min(512, NPAD - c0)
                    xts = xinp.tile([128, cw], b16, tag="xts", name="xts")
                    nc.sync.dma_start(xts[:], d_xt[:, c0:c0 + cw])
                    ps = psxw.tile([128, cw], f32, tag="psxw", name="ps1")
                    nc.tensor.matmul(ps[:], lhsT=w0[:], rhs=xts[:],
                                     start=True, stop=True)
                    nc.scalar.activation(out=h[:, c0:c0 + cw], in_=ps[:],
                                         func=Relu, bias=b0[:])
                    emit_phase_a_chunk(0, k)
                    for q in _AGQ_AT_K.get(k, []):
                        emit_ag(q, 0)

                # ---- GCN layers (phase A of l+1 and pooling are emitted
                # inside the epilogues) ----
                for l in range(L):
                    emit_phase_b(l)

                # ---- global add pool: AllReduce partials ----
                pool_sb = clsp.tile([G, 128], f32, tag="poolsb", name="poolsb")
                nc.vector.tensor_copy(out=pool_sb[:], in_=pool_state["psp"][:])
                nc.sync.dma_start(pool_in[:], pool_sb[:])
                if "noar" in ablate:
                    nc.sync.dma_start(
                        bass.AP(pool_out, 0, pool_in[:].ap), pool_in[:])
                else:
                    nc.gpsimd.collective_compute(
                        "AllReduce", AT.add, ins=[pool_in[:]],
                        outs=[pool_out[:]], replica_groups=rg)

                # ---- classifier (replicated) ----
                pooled_f = clsp.tile([G, 128], f32, tag="pooledf",
                                     name="pooledf")
                nc.sync.dma_start(pooled_f[:], pool_out[:])
                pooled_b = clsp.tile([G, 128], b16, tag="pooledb",
                                     name="pooledb")
                nc.vector.tensor_copy(out=pooled_b[:], in_=pooled_f[:])
                pstp = pstr.tile([128, G], b16, tag="pstr", name="pstC")
                nc.tensor.transpose(pstp[:], pooled_b[:], idg[:])
                pooledT = clsp.tile([128, G], b16, tag="pooledT",
                                    name="pooledT")
                nc.vector.tensor_copy(out=pooledT[:], in_=pstp[:])
                psz = pstr.tile([128, G], f32, tag="pstr", name="psz")
                nc.tensor.matmul(psz[:], lhsT=wc1[:], rhs=pooledT[:],
                                 start=True, stop=True)
                zt = clsp.tile([128, G], b16, tag="zt", name="zt")
                nc.scalar.activation(out=zt[:], in_=psz[:], func=Relu,
                                     bias=bc1[:])
                pslg = pstr.tile([G, C], f32, tag="pstr", name="pslg")
                nc.tensor.matmul(pslg[:], lhsT=zt[:], rhs=wc2[:],
                                 start=True, stop=True)
                lg = clsp.tile([G, C], f32, tag="lg", name="lg")
                nc.vector.tensor_tensor(out=lg[:], in0=pslg[:], in1=bc2m[:],
                                        op=AT.add)
                # softmax over C (free dim)
                mx = clsp.tile([G, 1], f32, tag="mx", name="mx")
                nc.vector.tensor_reduce(out=mx[:], in_=lg[:],
                                        axis=mybir.AxisListType.X, op=AT.max)
                nmx = clsp.tile([G, 1], f32, tag="nmx", name="nmx")
                nc.vector.tensor_scalar_mul(nmx[:], mx[:], -1.0)
                ex = clsp.tile([G, C], f32, tag="ex", name="ex")
                nc.scalar.activation(out=ex[:], in_=lg[:], func=Exp,
                                     bias=nmx[:])
                sm = clsp.tile([G, 1], f32, tag="sm", name="sm")
                nc.vector.tensor_reduce(out=sm[:], in_=ex[:],
                                        axis=mybir.AxisListType.X, op=AT.add)
                rs = clsp.tile([G, 1], f32, tag="rs", name="rs")
                nc.vector.reciprocal(rs[:], sm[:])
                prob = clsp.tile([G, C], f32, tag="prob", name="prob")
                nc.vector.tensor_scalar_mul(prob[:], ex[:], rs[:])
                nc.sync.dma_start(d_out[:], prob[:])

    nc.compile()
    return nc


# --------------------------------------------------------------------------
# entry point
# --------------------------------------------------------------------------
def kernel(x, edge_index, batch, W0, b0, Wg, bg, Wc1, bc1, Wc2, bc2,
           **extra):
    x = np.asarray(x, np.float32)
    edge_index = np.asarray(edge_index)
    batch = np.asarray(batch)
    W0 = np.asarray(W0, np.float32)
    Wg = np.asarray(Wg, np.float32)
    L = Wg.shape[0]

    key = (x.shape, edge_index.shape,
           hash(edge_index.tobytes()), hash(np.asarray(batch).tobytes()))
    if key not in _cache:
        meta, data = _preprocess(x, edge_index, batch)
        nc = _build(meta, L)
        _cache.clear()
        _cache[key] = (meta, data, nc)
    meta, data, nc = _cache[key]

    iota16 = np.broadcast_to(np.arange(128, dtype=np.float16),
                             (128, 128)).copy()
    iotag = np.broadcast_to(np.arange(G, dtype=np.float32), (128, G)).copy()
    common = dict(
        w0=W0.astype(bf16).view(np.uint16),
        wg=Wg.astype(bf16).view(np.uint16),
        wc1=np.asarray(Wc1, np.float32).astype(bf16).view(np.uint16),
        wc2=np.asarray(Wc2, np.float32).astype(bf16).view(np.uint16),
        b0=np.asarray(b0, np.float32).reshape(128, 1),
        bg=np.asarray(bg, np.float32).reshape(L, 128, 1),
        bc1=np.asarray(bc1, np.float32).reshape(128, 1),
        bc2m=np.broadcast_to(np.asarray(bc2, np.float32), (G, C)).copy(),
        iota16=iota16.view(np.uint16), iotag=iotag,
        id128=np.eye(128, dtype=np.float32).astype(bf16).view(np.uint16),
        idg=np.eye(G, dtype=np.float32).astype(bf16).view(np.uint16),
    )
    in_maps = []
    for c in range(NCORES):
        m = dict(common)
        m["xt"] = data["xt"][c].view(np.uint16)
        m["dinvt"] = data["dinvt"][c].view(np.uint16)
        m["gidx"] = data["gidx"][c]
        m["dstrel"] = data["dstrel"][c].view(np.uint16)
        m["batchrel"] = data["batchrel"][c]
        in_maps.append(m)

    trace = os.environ.get("BASS_KERNEL_TRACE", "0") == "1"
    res = run_bass_kernel_spmd(nc, in_maps, list(range(NCORES)), trace=trace)
    kernel._last_exec_ns = res.exec_time_ns
    kernel._last_results = res
    return np.asarray(res.results[0]["out"], np.float32)


kernel._last_exec_ns = None
